# revision 1
# baseline (speedup 1.0000x reference)
"""Trainium2 Bass kernel for nn_DilatedContextAttentionModule (B=8, C=256, 64x64).

Reference, per batch element (N = 64*64 = 4096):
    g   = G xj + g_b 1^T;  th = T xi + t_b 1^T;  phi = P xj + p_b 1^T
    f   = th^T phi / N                      (N x N, linear -- NO softmax)
    y[c,n] = sum_m f[n,m] g[c,m]
    z   = W y + W_b 1^T + xi
    out = BatchNorm2d(z)                    (training-mode batch stats)

Algebraic collapse (associativity; exact because f is linear):
    y = (1/N) (g phi^T) th = (1/N) S th,      S: C x C
    z = (E' + I) xi + d 1^T
    E' = (1/N) W S T,   d = (1/N) W S t_b + W_b
    S  = g0 phi0^T + (G sxj + N g_b) p_b^T + g_b (P sxj)^T
         (g0 = G xj, phi0 = P xj, sxj = xj @ 1)
This cuts ~9.7 GMAC/batch to ~0.9 GMAC/batch (the headroom of the problem).

Mapping to the NeuronCore (one batch element per core, 8 cores):
  phase 1  conv + S:  per 128-column chunk of n, one PSUM group computes
           [g0^T | phi0^T] (lhsT = xj chunk, n lands on partitions -- no
           transposes anywhere in the kernel), ACT copies PSUM->SBUF as
           float32r, then two matmuls accumulate S in PSUM across all 32
           chunks; the two bias rank-1 terms are K=1 matmuls.
  phase 2  E'^T = T^T (S^T W^T/N) + I and d via small matmuls; identity
           added by DVE during the PSUM->SBUF move.
  phase 3  z tiles [128, 512] = E_aug^T.T @ xi (+ d x ones row, K=1);
           ACT copies PSUM->SBUF; DVE bn_stats per tile (mean/var).
  BN       per-channel (mean, mean-of-squares)/8 packed [128, 2] per
           channel-chunk; ONE AllReduce per chunk -- chunk 0's collective
           + normalize + store overlap chunk 1's compute, so only the
           second collective's ~10 us floor lands on the critical path.
  stores   normalize in-place (DVE tensor_scalar) and DMA out per half.

TensorE dtype: float32r (fp32 bits streamed at 1 cycle/row for moving
free dim >= 256, vs 4 cycles/row for plain fp32; ~13-14 effective
mantissa bits). All matmul operand tiles are allocated float32r; the
producers (casting gpsimd DMAs, ACT/DVE copies) emit rounded values as
the walrus verifier requires. Measured end-to-end rms relative error vs
the fp32 jax reference: 2.3e-4 (plain fp32 build: 8.6e-7, ~2x slower).

Cost-model timeline: 71.5 us/core (collective priced as a local copy);
realistic HW estimate ~80 us/core including one unhidden 8-core
AllReduce floor (~9.7 us).
"""

import numpy as np

import concourse.bass as bass
import concourse.bacc as bacc
import concourse.tile as tile
from concourse import mybir
from concourse import bass_utils

B = 8
C = 256
N = 4096          # 64 * 64
NCORES = 8
NCH = 2           # channel chunks of 128
NT = 32           # n chunks of 128 (phase 1)
NZ = 8            # n tiles of 512 (phase 3)
F32 = mybir.dt.float32
BN_EPS = 1e-5

# TensorE compute dtype for the big matmuls. float32r streams at
# 1 cycle/row (vs 4 for float32) when the moving free dim >= 256, but
# requires all producers to round their outputs to float32r.
import os as _os
MM_DT = {
    "f32": mybir.dt.float32,
    "f32r": mybir.dt.float32r,
    "bf16": mybir.dt.bfloat16,
}[_os.environ.get("DCAM_MM_DT", "f32r")]


def _mm(x: bass.AP) -> bass.AP:
    # Tiles feeding matmuls are allocated as MM_DT directly; no-op now.
    return x


def build_kernel(nc, skip_cc: bool = False) -> None:
    f32 = F32
    xi_d = nc.dram_tensor("xi", [C, N], f32, kind="ExternalInput").ap()
    xj_d = nc.dram_tensor("xj", [C, N], f32, kind="ExternalInput").ap()
    # [128, 2, 512]: packed per-chunk conv weights [G^T | P^T]
    wgp_d = nc.dram_tensor("wgp", [128, NCH, 512], f32, kind="ExternalInput").ap()
    # [128, 2, 256]: theta_w rows (lhsT for E'^T), chunked on cp
    wtw_d = nc.dram_tensor("wtw", [128, NCH, C], f32, kind="ExternalInput").ap()
    # [128, 2, 256]: (W_w^T / N) rows, chunked on cg
    wwt_d = nc.dram_tensor("wwt", [128, NCH, C], f32, kind="ExternalInput").ap()
    # [128, 2]: theta_b column, chunked
    wtb_d = nc.dram_tensor("wtb", [128, NCH], f32, kind="ExternalInput").ap()
    # [1, 1024]: rows [N*g_b | g_b | p_b | W_b]
    aux_d = nc.dram_tensor("aux", [1, 4 * C + 512], f32, kind="ExternalInput").ap()
    # [128, 2, 2]: (gamma, beta) per channel, chunked
    gbe_d = nc.dram_tensor("gbe", [128, NCH, 2], f32, kind="ExternalInput").ap()
    # [128, 2]: W_b column, chunked
    wbc_d = nc.dram_tensor("wbc", [128, NCH], f32, kind="ExternalInput").ap()
    # [128, 2, 256]: identity matrix chunks (for z = (E'+I) xi + d 1^T)
    idn_d = nc.dram_tensor("idn", [128, NCH, C], f32, kind="ExternalInput").ap()
    out_d = nc.dram_tensor("out", [C, N], f32, kind="ExternalOutput").ap()

    with tile.TileContext(nc) as tc:
        _body(tc, xi_d, xj_d, wgp_d, wtw_d, wwt_d, wtb_d, aux_d, gbe_d, idn_d,
              wbc_d, out_d, skip_cc=skip_cc)


def _body(tc, xi_d, xj_d, wgp_d, wtw_d, wwt_d, wtb_d, aux_d, gbe_d, idn_d,
          wbc_d, out_d, skip_cc: bool = False):
    nc = tc.nc
    f32 = F32
    import contextlib

    with contextlib.ExitStack() as ctx:
        constp = ctx.enter_context(tc.tile_pool(name="const", bufs=1))
        datap = ctx.enter_context(tc.tile_pool(name="data", bufs=1))
        workp = ctx.enter_context(tc.tile_pool(name="work", bufs=4))
        rowsp = ctx.enter_context(tc.tile_pool(name="rows", bufs=2))
        psbig = ctx.enter_context(tc.tile_pool(name="ps_big", bufs=3, space="PSUM"))
        psacc = ctx.enter_context(tc.tile_pool(name="ps_acc", bufs=2, space="PSUM"))
        pssml = ctx.enter_context(tc.tile_pool(name="ps_sml", bufs=1, space="PSUM"))
        dramp = ctx.enter_context(tc.tile_pool(name="dram", bufs=2, space="DRAM"))

        # ---- constants / weights ------------------------------------
        mdt = MM_DT
        NQ = 4
        HN = N // NQ
        w_gp = constp.tile([128, NCH, 512], mdt, tag="w_gp")
        nc.gpsimd.dma_start(out=w_gp, in_=wgp_d)
        xj_h = []
        for h in range(NQ):
            t = datap.tile([128, NCH, HN], mdt, tag=f"xjh{h}", name=f"xj_h{h}")
            nc.gpsimd.dma_start(
                out=t,
                in_=xj_d.rearrange("(k p) n -> p k n", p=128)[:, :, h * HN:(h + 1) * HN],
            )
            xj_h.append(t)
        w_tw = constp.tile([128, NCH, C], mdt, tag="w_tw")
        nc.gpsimd.dma_start(out=w_tw, in_=wtw_d)
        w_wt = constp.tile([128, NCH, C], mdt, tag="w_wt")
        nc.gpsimd.dma_start(out=w_wt, in_=wwt_d)
        w_tb = constp.tile([128, NCH], mdt, tag="w_tb")
        nc.gpsimd.dma_start(out=w_tb, in_=wtb_d)
        aux = constp.tile([1, 4 * C + 512], mdt, tag="aux")
        nc.gpsimd.dma_start(out=aux, in_=aux_d)
        gbe = constp.tile([128, NCH, 2], f32, tag="gbe")
        nc.sync.dma_start(out=gbe, in_=gbe_d)
        wbc = constp.tile([128, NCH], f32, tag="wbc")
        nc.sync.dma_start(out=wbc, in_=wbc_d)
        idn = constp.tile([128, NCH, C], mdt, tag="idn")
        nc.gpsimd.dma_start(out=idn, in_=idn_d)
        eps = constp.tile([128, 1], f32, tag="eps")
        nc.vector.memset(eps, BN_EPS)

        # ---- big data tiles -----------------------------------------
        XHN = N // 2
        xi_h = []
        for h in range(2):
            t = datap.tile([128, NCH, XHN], mdt, tag=f"xih{h}", name=f"xi_h{h}")
            nc.gpsimd.dma_start(
                out=t,
                in_=xi_d.rearrange("(k p) n -> p k n", p=128)[:, :, h * XHN:(h + 1) * XHN],
            )
            xi_h.append(t)

        def xi_sl(k, tix):
            # phase-3 tile tix of 512 columns, channel-chunk k
            h, off = divmod(tix * 512, XHN)
            return xi_h[h][:, k, off:off + 512]

        def xj_sl(k, i):
            # phase-1 chunk i of 128 columns, channel-chunk k
            h, off = divmod(i * 128, HN)
            return xj_h[h][:, k, off:off + 128]

        # ---- sxj = rowsum(xj); bias-correction rows ------------------
        sxj = rowsp.tile([128, NCH], mdt, tag="sxj")
        sxjp = rowsp.tile([128, NCH, NQ], f32, tag="sxjp")
        with nc.allow_low_precision(reason="f32r output carries full fp32 bits"):
            for k in range(NCH):
                for h in range(NQ):
                    nc.vector.reduce_sum(
                        out=sxjp[:, k, h:h + 1], in_=xj_h[h][:, k, :],
                        axis=mybir.AxisListType.X,
                    )
                nc.vector.reduce_sum(
                    out=sxj[:, k:k + 1], in_=sxjp[:, k, :],
                    axis=mybir.AxisListType.X,
                )
        # s_g0_row = sxj^T @ G^T, s_phi0_row = sxj^T @ P^T   (each [1, 256])
        srow_ps = pssml.tile([1, 2 * C], f32, tag="sml")
        for k in range(NCH):
            nc.tensor.matmul(
                srow_ps[:, 0:C],
                _mm(sxj[:, k:k + 1]),
                _mm(w_gp[:, k, 0:C]),
                start=(k == 0), stop=(k == NCH - 1),
            )
        for k in range(NCH):
            nc.tensor.matmul(
                srow_ps[:, C:2 * C],
                _mm(sxj[:, k:k + 1]),
                _mm(w_gp[:, k, C:2 * C]),
                start=(k == 0), stop=(k == NCH - 1),
            )
        # u_row = s_g0 + N*g_b ; v_row = s_phi0
        urow = rowsp.tile([1, C], mdt, tag="urow")
        nc.vector.tensor_add(urow, srow_ps[:, 0:C], aux[:, 0:C])
        vrow = rowsp.tile([1, C], mdt, tag="vrow")
        nc.vector.tensor_copy(vrow, srow_ps[:, C:2 * C])

        # ---- phase 1: S = g0 phi0^T (+ rank-1 bias corrections) -----
        S_ps = [psacc.tile([128, C], f32, tag="acc", name=f"S_ps{m}") for m in range(NCH)]
        for i in range(NT):
            gp_ps = psbig.tile([128, 512], f32, tag="big")
            for k in range(NCH):
                nc.tensor.matmul(
                    gp_ps, _mm(xj_sl(k, i)), _mm(w_gp[:, k, :]),
                    start=(k == 0), stop=(k == NCH - 1),
                )
            gpt = workp.tile([128, 512], mdt, tag="gpt")
            if i >= 24:
                # late chunks: sxj is done, DVE has slack; shorten ACT chain
                nc.vector.tensor_copy(gpt, gp_ps)
            else:
                nc.scalar.copy(gpt, gp_ps)
            for m in range(NCH):
                nc.tensor.matmul(
                    S_ps[m],
                    _mm(gpt[:, m * 128:(m + 1) * 128]),
                    _mm(gpt[:, C:2 * C]),
                    start=(i == 0), stop=False,
                )
        for m in range(NCH):
            msl = slice(m * 128, (m + 1) * 128)
            nc.tensor.matmul(
                S_ps[m], _mm(urow[:, msl]), _mm(aux[:, 2 * C:3 * C]),
                start=False, stop=False,
            )
            nc.tensor.matmul(
                S_ps[m], _mm(aux[:, C + m * 128:C + (m + 1) * 128]), _mm(vrow),
                start=False, stop=True,
            )
        S_sb = []
        for m in range(NCH):
            t = workp.tile([128, C], mdt, tag=f"S{m}")
            nc.vector.tensor_copy(t, S_ps[m])
            S_sb.append(t)

        # ---- phase 2: V = S^T (W^T/N);  E'^T = T^T V;  d = V^T t_b --
        V_sb = []
        for m in range(NCH):
            v_ps = psacc.tile([128, C], f32, tag="acc")
            msl = slice(m * 128, (m + 1) * 128)
            for k in range(NCH):
                nc.tensor.matmul(
                    v_ps, _mm(S_sb[k][:, msl]), _mm(w_wt[:, k, :]),
                    start=(k == 0), stop=(k == NCH - 1),
                )
            t = workp.tile([128, C], mdt, tag=f"V{m}")
            nc.vector.tensor_copy(t, v_ps)
            V_sb.append(t)
        ET_sb = []
        for m in range(NCH):
            e_ps = psacc.tile([128, C], f32, tag="acc")
            msl = slice(m * 128, (m + 1) * 128)
            for k in range(NCH):
                nc.tensor.matmul(
                    e_ps, _mm(w_tw[:, k, msl]), _mm(V_sb[k]),
                    start=(k == 0), stop=(k == NCH - 1),
                )
            t = workp.tile([128, C], mdt, tag=f"ET{m}")
            nc.vector.tensor_add(t, e_ps, idn[:, m, :])
            ET_sb.append(t)
        dcol_ps = pssml.tile([128, NCH], f32, tag="sml")
        for j in range(NCH):
            for k in range(NCH):
                # N=1 moving dim: f32r is not ISA-legal here, use plain f32
                nc.tensor.matmul(
                    dcol_ps[:, j:j + 1],
                    V_sb[k][:, j * 128:(j + 1) * 128].bitcast(F32),
                    w_tb[:, k:k + 1].bitcast(F32),
                    start=(k == 0), stop=(k == NCH - 1),
                )
        dcol = rowsp.tile([128, NCH], f32, tag="dcol")
        nc.vector.tensor_add(dcol, dcol_ps, wbc)

        # ---- phase 3: z = (E'+I)^T.T @ xi + d 1^T; BN stats fused ---
        z_t = datap.tile([128, NCH, N], f32, tag="z")
        spack = rowsp.tile([128, 4], f32, tag="spack")
        ssum = rowsp.tile([128, 4], f32, tag="ssum")
        for j in range(NCH):
            jsl = slice(j * 128, (j + 1) * 128)
            stats = workp.tile([128, NZ, 6], f32, tag="bnst", name=f"stats{j}")
            for tix in range(NZ):
                tsl = slice(tix * 512, (tix + 1) * 512)
                z_ps = psbig.tile([128, 512], f32, tag="big")
                for k in range(NCH):
                    nc.tensor.matmul(
                        z_ps, _mm(ET_sb[k][:, jsl]), _mm(xi_sl(k, tix)),
                        start=(k == 0), stop=(k == NCH - 1),
                    )
                nc.scalar.activation(
                    out=z_t[:, j, tsl], in_=z_ps,
                    func=mybir.ActivationFunctionType.Identity,
                    bias=dcol[:, j:j + 1], scale=1.0,
                )
                nc.vector.bn_stats(out=stats[:, tix, :], in_=z_t[:, j, tsl])
            mv = rowsp.tile([128, 2], f32, tag="mv")
            nc.vector.bn_aggr(out=mv, in_=stats)
            nc.vector.tensor_scalar_mul(
                spack[:, 2 * j:2 * j + 1], mv[:, 0:1], 1.0 / NCORES)
            # (mean^2 + var) / NCORES  (= mean of squares, pre-scaled)
            nc.vector.scalar_tensor_tensor(
                out=spack[:, 2 * j + 1:2 * j + 2], in0=mv[:, 0:1],
                scalar=mv[:, 0:1], in1=mv[:, 1:2],
                op0=mybir.AluOpType.mult, op1=mybir.AluOpType.add,
            )
            nc.vector.tensor_scalar_mul(
                spack[:, 2 * j + 1:2 * j + 2],
                spack[:, 2 * j + 1:2 * j + 2], 1.0 / NCORES)
            cc_in = dramp.tile([128, 2], f32, tag=f"cc_in{j}", name=f"cc_in{j}")
            cc_out = dramp.tile([128, 2], f32, tag=f"cc_out{j}", name=f"cc_out{j}")
            nc.sync.dma_start(out=cc_in, in_=spack[:, 2 * j:2 * j + 2])
            if skip_cc:
                nc.sync.dma_start(out=cc_out, in_=cc_in)
            else:
                nc.gpsimd.collective_compute(
                    "AllReduce",
                    mybir.AluOpType.add,
                    replica_groups=[list(range(NCORES))],
                    ins=[cc_in.opt()],
                    outs=[cc_out.opt()],
                )
            nc.sync.dma_start(out=ssum[:, 2 * j:2 * j + 2], in_=cc_out)

            # ---- normalize + affine + store (inside j loop: chunk 0's
            # collective + store overlap chunk 1's compute) ------------
            mcol = ssum[:, 2 * j:2 * j + 1]
            qcol = ssum[:, 2 * j + 1:2 * j + 2]
            # negvar = m^2 - q  (sqrt uses scale=-1 to flip the sign)
            nvcol = rowsp.tile([128, 1], f32, tag="nvcol")
            nc.vector.scalar_tensor_tensor(
                out=nvcol, in0=mcol, scalar=mcol, in1=qcol,
                op0=mybir.AluOpType.mult, op1=mybir.AluOpType.subtract,
            )
            # rstd = 1 / sqrt(-negvar + eps) = 1 / sqrt(var + eps)
            scol = rowsp.tile([128, 1], f32, tag="scol")
            nc.scalar.activation(
                out=scol, in_=nvcol, func=mybir.ActivationFunctionType.Sqrt,
                bias=eps, scale=-1.0,
            )
            nc.vector.reciprocal(out=scol, in_=scol)
            acol = rowsp.tile([128, 1], f32, tag="acol")
            nc.vector.tensor_mul(acol, scol, gbe[:, j, 0:1])
            # nbcol = m*a - beta;  apply computes z*a - nbcol = z*a + beta - m*a
            bcol = rowsp.tile([128, 1], f32, tag="bcol")
            nc.vector.scalar_tensor_tensor(
                out=bcol, in0=mcol, scalar=acol, in1=gbe[:, j, 1:2],
                op0=mybir.AluOpType.mult, op1=mybir.AluOpType.subtract,
            )
            # apply z*a - nb in halves, each half split DVE || ACT so the
            # post-collective tail is half as long
            nbcol = rowsp.tile([128, 1], f32, tag="nbcol")
            nc.vector.tensor_scalar_mul(nbcol, bcol, -1.0)
            for h in range(2):
                hsl = slice(h * (N // 2), (h + 1) * (N // 2))
                q0 = slice(h * (N // 2), h * (N // 2) + N // 4)
                q1 = slice(h * (N // 2) + N // 4, (h + 1) * (N // 2))
                nc.vector.tensor_scalar(
                    out=z_t[:, j, q0], in0=z_t[:, j, q0],
                    scalar1=acol, scalar2=bcol,
                    op0=mybir.AluOpType.mult, op1=mybir.AluOpType.subtract,
                )
                nc.scalar.activation(
                    out=z_t[:, j, q1], in_=z_t[:, j, q1],
                    func=mybir.ActivationFunctionType.Identity,
                    bias=nbcol, scale=acol,
                )
                nc.sync.dma_start(
                    out=out_d[j * 128:(j + 1) * 128, hsl], in_=z_t[:, j, hsl])


_NC_CACHE: dict = {}


def _get_nc():
    if "nc" not in _NC_CACHE:
        nc = bacc.Bacc(
            "TRN2",
            target_bir_lowering=False,
            debug=False,
            enable_asserts=True,
            num_devices=NCORES,
        )
        build_kernel(nc)
        nc.compile()
        _NC_CACHE["nc"] = nc
    return _NC_CACHE["nc"]


def _make_in_maps(inputs: dict) -> list[dict]:
    xi = np.ascontiguousarray(np.asarray(inputs["xi"], np.float32).reshape(B, C, N))
    xj = np.ascontiguousarray(np.asarray(inputs["xj"], np.float32).reshape(B, C, N))
    g_w = np.asarray(inputs["g_w"], np.float32)
    g_b = np.asarray(inputs["g_b"], np.float32)
    t_w = np.asarray(inputs["theta_w"], np.float32)
    t_b = np.asarray(inputs["theta_b"], np.float32)
    p_w = np.asarray(inputs["phi_w"], np.float32)
    p_b = np.asarray(inputs["phi_b"], np.float32)
    W_w = np.asarray(inputs["W_w"], np.float32)
    W_b = np.asarray(inputs["W_b"], np.float32)
    gam = np.asarray(inputs["bn_gamma"], np.float32)
    bet = np.asarray(inputs["bn_beta"], np.float32)

    def chunked(a):  # [256, F] -> [128, 2, F]
        return np.ascontiguousarray(a.reshape(2, 128, -1).transpose(1, 0, 2))

    wgp = chunked(np.concatenate([g_w.T, p_w.T], axis=1))          # [128,2,512]
    wtw = chunked(t_w)                                             # [128,2,256]
    wwt = chunked(W_w.T * (1.0 / N))                               # [128,2,256]
    wtb = np.ascontiguousarray(t_b.reshape(2, 128).T)              # [128,2]
    aux = np.concatenate([N * g_b, g_b, p_b, W_b,
                          np.ones(512, np.float32)])[None, :]   # [1,1536]
    aux = np.ascontiguousarray(aux.astype(np.float32))
    gbe = chunked(np.stack([gam, bet], axis=1))                    # [128,2,2]
    idn = chunked(np.eye(C, dtype=np.float32))                     # [128,2,256]
    wbc = np.ascontiguousarray(W_b.reshape(2, 128).T)              # [128,2]

    in_maps = []
    for b in range(B):
        in_maps.append({
            "xi": xi[b], "xj": xj[b],
            "wgp": wgp, "wtw": wtw, "wwt": wwt, "wtb": wtb,
            "aux": aux, "gbe": gbe, "idn": idn, "wbc": wbc,
        })
    return in_maps


def kernel(**inputs) -> np.ndarray:
    nc = _get_nc()
    in_maps = _make_in_maps(inputs)
    last_err = None
    for attempt in range(3):
        try:
            res = bass_utils.run_bass_kernel_spmd(
                nc, in_maps, core_ids=list(range(NCORES)),
            )
            break
        except Exception as e:  # transient device wedge: back off and retry
            last_err = e
            import time as _time
            _time.sleep(4.0 * (attempt + 1))
            try:
                import jax
                import jax.extend.backend as _jeb
                jax.clear_caches()
                # tear down the PJRT client: a fresh axon connection lets the
                # terminal reset a wedged exec unit
                _jeb.clear_backends()
            except Exception:
                pass
    else:
        raise last_err
    out = np.stack([res.results[c]["out"] for c in range(NCORES)])
    return np.ascontiguousarray(out.reshape(B, C, 64, 64).astype(np.float32))


if __name__ == "__main__":
    rng = np.random.default_rng(0)
    fake = {
        "xi": rng.standard_normal((B, C, 64, 64), np.float32),
        "xj": rng.standard_normal((B, C, 64, 64), np.float32),
        "g_w": rng.standard_normal((C, C), np.float32) / 16,
        "g_b": rng.standard_normal((C,), np.float32) / 16,
        "theta_w": rng.standard_normal((C, C), np.float32) / 16,
        "theta_b": rng.standard_normal((C,), np.float32) / 16,
        "phi_w": rng.standard_normal((C, C), np.float32) / 16,
        "phi_b": rng.standard_normal((C,), np.float32) / 16,
        "W_w": rng.standard_normal((C, C), np.float32) / 16,
        "W_b": rng.standard_normal((C,), np.float32) / 16,
        "bn_gamma": np.ones((C,), np.float32),
        "bn_beta": np.zeros((C,), np.float32),
    }
    out = kernel(**fake)
    print("out", out.shape, out.dtype, float(np.abs(out).mean()))



# revision 3
# speedup vs baseline: 1.3265x; 1.3265x over previous
"""Trainium2 Bass kernel for nn_DilatedContextAttentionModule (B=8, C=256, 64x64).

Reference, per batch element (N = 64*64 = 4096):
    g   = G xj + g_b 1^T;  th = T xi + t_b 1^T;  phi = P xj + p_b 1^T
    f   = th^T phi / N                      (N x N, linear -- NO softmax)
    y[c,n] = sum_m f[n,m] g[c,m]
    z   = W y + W_b 1^T + xi
    out = BatchNorm2d(z)                    (training-mode batch stats)

Algebraic collapse (associativity; exact because f is linear):
    y = (1/N) (g phi^T) th = (1/N) S th,      S: C x C
    z = (E' + I) xi + d 1^T
    E' = (1/N) W S T,   d = (1/N) W S t_b + W_b
    S  = g0 phi0^T + (G sxj + N g_b) p_b^T + g_b (P sxj)^T
         (g0 = G xj, phi0 = P xj, sxj = xj @ 1)

Mapping to the NeuronCore (one batch element per core, 8 cores):
  phase 1  conv + S:  per 128-column chunk of n, one PSUM group computes
           [g0^T | phi0^T] (lhsT = xj chunk, n lands on partitions), a
           copy moves PSUM->SBUF (bf16), then two matmuls accumulate S in
           PSUM across all 32 chunks; the two bias rank-1 terms are K=1
           matmuls.
  phase 2  E'^T = T^T (S^T W^T/N) + I and d via small matmuls.
  phase 3  z tiles [128, 512] = E_aug^T.T @ xi (+ d bias via ACT);
           DVE bn_stats per tile (mean/var).
  BN       per-channel (mean, mean-of-squares)/8 packed [128, 4] for BOTH
           channel chunks; ONE AllGather (out [8, 512]) + local 3-add
           reduction replaces the two serialized AllReduces (an AllGather
           has no reduce multiplier on the fabric and one launch overhead
           instead of two).
  stores   normalize in-place (DVE/ACT split) and DMA out per half.

Compute dtype: bf16 end-to-end for matmul operands (PE streams bf16 at
1 cycle/row like f32r; DMA bytes halve; inputs are cast to bf16 on the
host so the wire traffic is bf16 too). PSUM accumulation stays fp32.
Measured rms relative error vs the fp32 jax reference: ~2e-3 (well
inside the 2e-2 gate).
"""

import numpy as np

import concourse.bass as bass
import concourse.bacc as bacc
import concourse.tile as tile
from concourse import mybir
from concourse import bass_utils

B = 8
C = 256
N = 4096          # 64 * 64
NCORES = 8
NCH = 2           # channel chunks of 128
NT = 32           # n chunks of 128 (phase 1)
NZ = 8            # n tiles of 512 (phase 3)
NQJ = 8           # xj DMA pieces (512 cols each)
NQI = 4           # xi DMA pieces (1024 cols each)
F32 = mybir.dt.float32
BF16 = mybir.dt.bfloat16
BN_EPS = 1e-5

MM_DT = BF16


def build_kernel(nc, skip_cc: bool = False) -> None:
    f32 = F32
    xi_d = nc.dram_tensor("xi", [C, N], MM_DT, kind="ExternalInput").ap()
    xj_d = nc.dram_tensor("xj", [C, N], MM_DT, kind="ExternalInput").ap()
    # [128, 2, 512]: packed per-chunk conv weights [G^T | P^T]
    wgp_d = nc.dram_tensor("wgp", [128, NCH, 512], MM_DT, kind="ExternalInput").ap()
    # [128, 2, 256]: theta_w rows (lhsT for E'^T), chunked on cp
    wtw_d = nc.dram_tensor("wtw", [128, NCH, C], MM_DT, kind="ExternalInput").ap()
    # [128, 2, 256]: (W_w^T / N) rows, chunked on cg
    wwt_d = nc.dram_tensor("wwt", [128, NCH, C], MM_DT, kind="ExternalInput").ap()
    # [128, 2]: theta_b column, chunked
    wtb_d = nc.dram_tensor("wtb", [128, NCH], MM_DT, kind="ExternalInput").ap()
    # [1, 1024]: rows [N*g_b | g_b | p_b | W_b]
    aux_d = nc.dram_tensor("aux", [1, 4 * C], MM_DT, kind="ExternalInput").ap()
    # [128, 2, 2]: (gamma, beta) per channel, chunked
    gbe_d = nc.dram_tensor("gbe", [128, NCH, 2], f32, kind="ExternalInput").ap()
    # [128, 2]: W_b column, chunked
    wbc_d = nc.dram_tensor("wbc", [128, NCH], f32, kind="ExternalInput").ap()
    # [128, 2, 256]: identity matrix chunks (for z = (E'+I) xi + d 1^T)
    idn_d = nc.dram_tensor("idn", [128, NCH, C], MM_DT, kind="ExternalInput").ap()
    out_d = nc.dram_tensor("out", [C, N], f32, kind="ExternalOutput").ap()

    with tile.TileContext(nc) as tc:
        _body(tc, xi_d, xj_d, wgp_d, wtw_d, wwt_d, wtb_d, aux_d, gbe_d, idn_d,
              wbc_d, out_d, skip_cc=skip_cc)


def _body(tc, xi_d, xj_d, wgp_d, wtw_d, wwt_d, wtb_d, aux_d, gbe_d, idn_d,
          wbc_d, out_d, skip_cc: bool = False):
    nc = tc.nc
    f32 = F32
    import contextlib

    with contextlib.ExitStack() as ctx:
        constp = ctx.enter_context(tc.tile_pool(name="const", bufs=1))
        datap = ctx.enter_context(tc.tile_pool(name="data", bufs=1))
        workp = ctx.enter_context(tc.tile_pool(name="work", bufs=4))
        rowsp = ctx.enter_context(tc.tile_pool(name="rows", bufs=2))
        psbig = ctx.enter_context(tc.tile_pool(name="ps_big", bufs=3, space="PSUM"))
        psacc = ctx.enter_context(tc.tile_pool(name="ps_acc", bufs=2, space="PSUM"))
        pssml = ctx.enter_context(tc.tile_pool(name="ps_sml", bufs=1, space="PSUM"))
        dramp = ctx.enter_context(tc.tile_pool(name="dram", bufs=2, space="DRAM"))

        mdt = MM_DT
        # ---- weights needed by phase 1 first, then streaming data ----
        w_gp = constp.tile([128, NCH, 512], mdt, tag="w_gp")
        nc.gpsimd.dma_start(out=w_gp, in_=wgp_d)
        JW = N // NQJ
        xj_h = []
        for h in range(NQJ):
            t = datap.tile([128, NCH, JW], mdt, tag=f"xjh{h}", name=f"xj_h{h}")
            nc.gpsimd.dma_start(
                out=t,
                in_=xj_d.rearrange("(k p) n -> p k n", p=128)[:, :, h * JW:(h + 1) * JW],
            )
            xj_h.append(t)
        IW = N // NQI
        xi_h = []
        for h in range(NQI):
            t = datap.tile([128, NCH, IW], mdt, tag=f"xih{h}", name=f"xi_h{h}")
            nc.gpsimd.dma_start(
                out=t,
                in_=xi_d.rearrange("(k p) n -> p k n", p=128)[:, :, h * IW:(h + 1) * IW],
            )
            xi_h.append(t)
        # ---- phase-2 weights and small constants (after the big loads) ----
        w_tw = constp.tile([128, NCH, C], mdt, tag="w_tw")
        nc.gpsimd.dma_start(out=w_tw, in_=wtw_d)
        w_wt = constp.tile([128, NCH, C], mdt, tag="w_wt")
        nc.gpsimd.dma_start(out=w_wt, in_=wwt_d)
        w_tb = constp.tile([128, NCH], mdt, tag="w_tb")
        nc.gpsimd.dma_start(out=w_tb, in_=wtb_d)
        aux = constp.tile([1, 4 * C], mdt, tag="aux")
        nc.gpsimd.dma_start(out=aux, in_=aux_d)
        idn = constp.tile([128, NCH, C], mdt, tag="idn")
        nc.gpsimd.dma_start(out=idn, in_=idn_d)
        gbe = constp.tile([128, NCH, 2], f32, tag="gbe")
        nc.sync.dma_start(out=gbe, in_=gbe_d)
        wbc = constp.tile([128, NCH], f32, tag="wbc")
        nc.sync.dma_start(out=wbc, in_=wbc_d)
        eps = constp.tile([128, 1], f32, tag="eps")
        nc.vector.memset(eps, BN_EPS)

        def xi_sl(k, tix):
            # phase-3 tile tix of 512 columns, channel-chunk k
            h, off = divmod(tix * 512, IW)
            return xi_h[h][:, k, off:off + 512]

        def xj_sl(k, i):
            # phase-1 chunk i of 128 columns, channel-chunk k
            h, off = divmod(i * 128, JW)
            return xj_h[h][:, k, off:off + 128]

        # ---- sxj = rowsum(xj); bias-correction rows ------------------
        sxj = rowsp.tile([128, NCH], mdt, tag="sxj")
        sxjp = rowsp.tile([128, NCH, NQJ], f32, tag="sxjp")
        with nc.allow_low_precision(reason="bias-correction rank-1 terms"):
            for k in range(NCH):
                for h in range(NQJ):
                    nc.vector.reduce_sum(
                        out=sxjp[:, k, h:h + 1], in_=xj_h[h][:, k, :],
                        axis=mybir.AxisListType.X,
                    )
                nc.vector.reduce_sum(
                    out=sxj[:, k:k + 1], in_=sxjp[:, k, :],
                    axis=mybir.AxisListType.X,
                )
        # s_g0_row = sxj^T @ G^T, s_phi0_row = sxj^T @ P^T   (each [1, 256])
        srow_ps = pssml.tile([1, 2 * C], f32, tag="sml")
        for k in range(NCH):
            nc.tensor.matmul(
                srow_ps[:, 0:C],
                sxj[:, k:k + 1],
                w_gp[:, k, 0:C],
                start=(k == 0), stop=(k == NCH - 1),
            )
        for k in range(NCH):
            nc.tensor.matmul(
                srow_ps[:, C:2 * C],
                sxj[:, k:k + 1],
                w_gp[:, k, C:2 * C],
                start=(k == 0), stop=(k == NCH - 1),
            )
        # u_row = s_g0 + N*g_b ; v_row = s_phi0
        urow = rowsp.tile([1, C], mdt, tag="urow")
        nc.vector.tensor_add(urow, srow_ps[:, 0:C], aux[:, 0:C])
        vrow = rowsp.tile([1, C], mdt, tag="vrow")
        nc.vector.tensor_copy(vrow, srow_ps[:, C:2 * C])

        # ---- phase 1: S = g0 phi0^T (+ rank-1 bias corrections) -----
        S_ps = [psacc.tile([128, C], f32, tag="acc", name=f"S_ps{m}") for m in range(NCH)]
        for i in range(NT):
            gp_ps = psbig.tile([128, 512], f32, tag="big")
            for k in range(NCH):
                nc.tensor.matmul(
                    gp_ps, xj_sl(k, i), w_gp[:, k, :],
                    start=(k == 0), stop=(k == NCH - 1),
                )
            gpt = workp.tile([128, 512], mdt, tag="gpt")
            if i % 2 == 0:
                nc.scalar.copy(gpt, gp_ps)
            else:
                nc.vector.tensor_copy(gpt, gp_ps)
            for m in range(NCH):
                nc.tensor.matmul(
                    S_ps[m],
                    gpt[:, m * 128:(m + 1) * 128],
                    gpt[:, C:2 * C],
                    start=(i == 0), stop=False,
                )
        for m in range(NCH):
            msl = slice(m * 128, (m + 1) * 128)
            nc.tensor.matmul(
                S_ps[m], urow[:, msl], aux[:, 2 * C:3 * C],
                start=False, stop=False,
            )
            nc.tensor.matmul(
                S_ps[m], aux[:, C + m * 128:C + (m + 1) * 128], vrow,
                start=False, stop=True,
            )
        S_sb = []
        for m in range(NCH):
            t = workp.tile([128, C], mdt, tag=f"S{m}")
            nc.vector.tensor_copy(t, S_ps[m])
            S_sb.append(t)

        # ---- phase 2: V = S^T (W^T/N);  E'^T = T^T V;  d = V^T t_b --
        V_sb = []
        for m in range(NCH):
            v_ps = psacc.tile([128, C], f32, tag="acc")
            msl = slice(m * 128, (m + 1) * 128)
            for k in range(NCH):
                nc.tensor.matmul(
                    v_ps, S_sb[k][:, msl], w_wt[:, k, :],
                    start=(k == 0), stop=(k == NCH - 1),
                )
            t = workp.tile([128, C], mdt, tag=f"V{m}")
            nc.vector.tensor_copy(t, v_ps)
            V_sb.append(t)
        ET_sb = []
        for m in range(NCH):
            e_ps = psacc.tile([128, C], f32, tag="acc")
            msl = slice(m * 128, (m + 1) * 128)
            for k in range(NCH):
                nc.tensor.matmul(
                    e_ps, w_tw[:, k, msl], V_sb[k],
                    start=(k == 0), stop=(k == NCH - 1),
                )
            t = workp.tile([128, C], mdt, tag=f"ET{m}")
            nc.vector.tensor_add(t, e_ps, idn[:, m, :])
            ET_sb.append(t)
        dcol_ps = pssml.tile([128, NCH], f32, tag="sml")
        for j in range(NCH):
            for k in range(NCH):
                nc.tensor.matmul(
                    dcol_ps[:, j:j + 1],
                    V_sb[k][:, j * 128:(j + 1) * 128],
                    w_tb[:, k:k + 1],
                    start=(k == 0), stop=(k == NCH - 1),
                )
        dcol = rowsp.tile([128, NCH], f32, tag="dcol")
        nc.vector.tensor_add(dcol, dcol_ps, wbc)

        # ---- phase 3: z = (E'+I)^T.T @ xi + d 1^T; BN stats fused ---
        z_t = datap.tile([128, NCH, N], f32, tag="z")
        spack = rowsp.tile([128, 4], f32, tag="spack")
        for j in range(NCH):
            jsl = slice(j * 128, (j + 1) * 128)
            stats = workp.tile([128, NZ, 6], f32, tag="bnst", name=f"stats{j}")
            for tix in range(NZ):
                tsl = slice(tix * 512, (tix + 1) * 512)
                z_ps = psbig.tile([128, 512], f32, tag="big")
                for k in range(NCH):
                    nc.tensor.matmul(
                        z_ps, ET_sb[k][:, jsl], xi_sl(k, tix),
                        start=(k == 0), stop=(k == NCH - 1),
                    )
                nc.scalar.activation(
                    out=z_t[:, j, tsl], in_=z_ps,
                    func=mybir.ActivationFunctionType.Identity,
                    bias=dcol[:, j:j + 1], scale=1.0,
                )
                nc.vector.bn_stats(out=stats[:, tix, :], in_=z_t[:, j, tsl])
            mv = rowsp.tile([128, 2], f32, tag="mv")
            nc.vector.bn_aggr(out=mv, in_=stats)
            nc.vector.tensor_scalar_mul(
                spack[:, 2 * j:2 * j + 1], mv[:, 0:1], 1.0 / NCORES)
            # (mean^2 + var) / NCORES  (= mean of squares, pre-scaled)
            nc.vector.scalar_tensor_tensor(
                out=spack[:, 2 * j + 1:2 * j + 2], in0=mv[:, 0:1],
                scalar=mv[:, 0:1], in1=mv[:, 1:2],
                op0=mybir.AluOpType.mult, op1=mybir.AluOpType.add,
            )
            nc.vector.tensor_scalar_mul(
                spack[:, 2 * j + 1:2 * j + 2],
                spack[:, 2 * j + 1:2 * j + 2], 1.0 / NCORES)

        # ---- ONE AllGather for both chunks' stats; local 8-way sum --
        cc_in = dramp.tile([128, 4], f32, tag="cc_in", name="cc_in")
        cc_out = dramp.tile([NCORES, 128, 4], f32, tag="cc_out", name="cc_out")
        nc.sync.dma_start(out=cc_in, in_=spack)
        if skip_cc:
            nc.sync.dma_start(out=cc_out[0, :, :], in_=cc_in)
        else:
            nc.gpsimd.collective_compute(
                "AllGather",
                mybir.AluOpType.bypass,
                replica_groups=[list(range(NCORES))],
                ins=[cc_in.opt()],
                outs=[cc_out.opt()],
            )
        sall = rowsp.tile([128, NCORES, 4], f32, tag="sall")
        nc.sync.dma_start(
            out=sall, in_=cc_out.rearrange("r p s -> p r s"))
        s4 = rowsp.tile([128, 4, 4], f32, tag="s4")
        nc.vector.tensor_add(s4, sall[:, 0:4, :], sall[:, 4:8, :])
        s2 = rowsp.tile([128, 2, 4], f32, tag="s2")
        nc.vector.tensor_add(s2, s4[:, 0:2, :], s4[:, 2:4, :])
        ssum = rowsp.tile([128, 4], f32, tag="ssum")
        nc.vector.tensor_add(ssum, s2[:, 0, :], s2[:, 1, :])

        # ---- normalize + affine + store -----------------------------
        for j in range(NCH):
            mcol = ssum[:, 2 * j:2 * j + 1]
            qcol = ssum[:, 2 * j + 1:2 * j + 2]
            # negvar = m^2 - q  (sqrt uses scale=-1 to flip the sign)
            nvcol = rowsp.tile([128, 1], f32, tag="nvcol")
            nc.vector.scalar_tensor_tensor(
                out=nvcol, in0=mcol, scalar=mcol, in1=qcol,
                op0=mybir.AluOpType.mult, op1=mybir.AluOpType.subtract,
            )
            # rstd = 1 / sqrt(-negvar + eps) = 1 / sqrt(var + eps)
            scol = rowsp.tile([128, 1], f32, tag="scol")
            nc.scalar.activation(
                out=scol, in_=nvcol, func=mybir.ActivationFunctionType.Sqrt,
                bias=eps, scale=-1.0,
            )
            nc.vector.reciprocal(out=scol, in_=scol)
            acol = rowsp.tile([128, 1], f32, tag="acol")
            nc.vector.tensor_mul(acol, scol, gbe[:, j, 0:1])
            # nbcol = m*a - beta;  apply computes z*a - nbcol = z*a + beta - m*a
            bcol = rowsp.tile([128, 1], f32, tag="bcol")
            nc.vector.scalar_tensor_tensor(
                out=bcol, in0=mcol, scalar=acol, in1=gbe[:, j, 1:2],
                op0=mybir.AluOpType.mult, op1=mybir.AluOpType.subtract,
            )
            # apply z*a - nb in halves, each half split DVE || ACT so the
            # post-collective tail is short
            nbcol = rowsp.tile([128, 1], f32, tag="nbcol")
            nc.vector.tensor_scalar_mul(nbcol, bcol, -1.0)
            for h in range(2):
                hsl = slice(h * (N // 2), (h + 1) * (N // 2))
                q0 = slice(h * (N // 2), h * (N // 2) + N // 4)
                q1 = slice(h * (N // 2) + N // 4, (h + 1) * (N // 2))
                nc.vector.tensor_scalar(
                    out=z_t[:, j, q0], in0=z_t[:, j, q0],
                    scalar1=acol, scalar2=bcol,
                    op0=mybir.AluOpType.mult, op1=mybir.AluOpType.subtract,
                )
                nc.scalar.activation(
                    out=z_t[:, j, q1], in_=z_t[:, j, q1],
                    func=mybir.ActivationFunctionType.Identity,
                    bias=nbcol, scale=acol,
                )
                nc.sync.dma_start(
                    out=out_d[j * 128:(j + 1) * 128, hsl], in_=z_t[:, j, hsl])


_NC_CACHE: dict = {}


def _get_nc():
    if "nc" not in _NC_CACHE:
        nc = bacc.Bacc(
            "TRN2",
            target_bir_lowering=False,
            debug=False,
            enable_asserts=True,
            num_devices=NCORES,
        )
        build_kernel(nc)
        nc.compile()
        _NC_CACHE["nc"] = nc
    return _NC_CACHE["nc"]


def _make_in_maps(inputs: dict) -> list[dict]:
    import ml_dtypes
    bf16 = ml_dtypes.bfloat16
    xi = np.ascontiguousarray(
        np.asarray(inputs["xi"], np.float32).reshape(B, C, N).astype(bf16))
    xj = np.ascontiguousarray(
        np.asarray(inputs["xj"], np.float32).reshape(B, C, N).astype(bf16))
    g_w = np.asarray(inputs["g_w"], np.float32)
    g_b = np.asarray(inputs["g_b"], np.float32)
    t_w = np.asarray(inputs["theta_w"], np.float32)
    t_b = np.asarray(inputs["theta_b"], np.float32)
    p_w = np.asarray(inputs["phi_w"], np.float32)
    p_b = np.asarray(inputs["phi_b"], np.float32)
    W_w = np.asarray(inputs["W_w"], np.float32)
    W_b = np.asarray(inputs["W_b"], np.float32)
    gam = np.asarray(inputs["bn_gamma"], np.float32)
    bet = np.asarray(inputs["bn_beta"], np.float32)

    def chunked(a):  # [256, F] -> [128, 2, F]
        return np.ascontiguousarray(a.reshape(2, 128, -1).transpose(1, 0, 2))

    wgp = chunked(np.concatenate([g_w.T, p_w.T], axis=1)).astype(bf16)
    wtw = chunked(t_w).astype(bf16)                                # [128,2,256]
    wwt = chunked(W_w.T * (1.0 / N)).astype(bf16)                  # [128,2,256]
    wtb = np.ascontiguousarray(t_b.reshape(2, 128).T).astype(bf16)  # [128,2]
    aux = np.concatenate([N * g_b, g_b, p_b, W_b])[None, :]
    aux = np.ascontiguousarray(aux.astype(bf16))                   # [1,1024]
    gbe = chunked(np.stack([gam, bet], axis=1))                    # [128,2,2]
    idn = chunked(np.eye(C, dtype=np.float32)).astype(bf16)        # [128,2,256]
    wbc = np.ascontiguousarray(W_b.reshape(2, 128).T)              # [128,2]

    in_maps = []
    for b in range(B):
        in_maps.append({
            "xi": xi[b], "xj": xj[b],
            "wgp": wgp, "wtw": wtw, "wwt": wwt, "wtb": wtb,
            "aux": aux, "gbe": gbe, "idn": idn, "wbc": wbc,
        })
    return in_maps


def kernel(**inputs) -> np.ndarray:
    nc = _get_nc()
    in_maps = _make_in_maps(inputs)
    last_err = None
    for attempt in range(3):
        try:
            res = bass_utils.run_bass_kernel_spmd(
                nc, in_maps, core_ids=list(range(NCORES)),
            )
            break
        except Exception as e:  # transient device wedge: back off and retry
            last_err = e
            import time as _time
            _time.sleep(4.0 * (attempt + 1))
            try:
                import jax
                import jax.extend.backend as _jeb
                jax.clear_caches()
                # tear down the PJRT client: a fresh axon connection lets the
                # terminal reset a wedged exec unit
                _jeb.clear_backends()
            except Exception:
                pass
    else:
        raise last_err
    out = np.stack([res.results[c]["out"] for c in range(NCORES)])
    return np.ascontiguousarray(out.reshape(B, C, 64, 64).astype(np.float32))


if __name__ == "__main__":
    rng = np.random.default_rng(0)
    fake = {
        "xi": rng.standard_normal((B, C, 64, 64), np.float32),
        "xj": rng.standard_normal((B, C, 64, 64), np.float32),
        "g_w": rng.standard_normal((C, C), np.float32) / 16,
        "g_b": rng.standard_normal((C,), np.float32) / 16,
        "theta_w": rng.standard_normal((C, C), np.float32) / 16,
        "theta_b": rng.standard_normal((C,), np.float32) / 16,
        "phi_w": rng.standard_normal((C, C), np.float32) / 16,
        "phi_b": rng.standard_normal((C,), np.float32) / 16,
        "W_w": rng.standard_normal((C, C), np.float32) / 16,
        "W_b": rng.standard_normal((C,), np.float32) / 16,
        "bn_gamma": np.ones((C,), np.float32),
        "bn_beta": np.zeros((C,), np.float32),
    }
    out = kernel(**fake)
    print("out", out.shape, out.dtype, float(np.abs(out).mean()))


# revision 74
# speedup vs baseline: 1.7334x; 1.3068x over previous
"""Trainium2 Bass kernel for nn_DilatedContextAttentionModule (B=8, C=256, 64x64).

Reference, per batch element (N = 64*64 = 4096):
    g   = G xj + g_b 1^T;  th = T xi + t_b 1^T;  phi = P xj + p_b 1^T
    f   = th^T phi / N                      (N x N, linear -- NO softmax)
    y[c,n] = sum_m f[n,m] g[c,m]
    z   = W y + W_b 1^T + xi
    out = BatchNorm2d(z)                    (training-mode batch stats)

Algebraic collapse (associativity; exact because f is linear):
    z  = (E' + I) xi + d 1^T,   E' = (1/N) W S T,  S = g phi^T
    with S = (G Xj)(P Xj)^T + u p_b^T + g_b v^T,
         u = G sxj + N g_b,  v = P sxj,  sxj = Xj 1.
Gram-form for the phase-2 matrix V := S^T (W/N)^T:
    V = P Gj (G^T W^T)/N + p_b (W'u)^T + v (W' g_b)^T,   Gj = Xj Xj^T.
All weight-by-weight products (G^T W^T, W G, P^T, ...) are precomputed on
the HOST, so the device never runs the 536 MMAC "conv": it computes the
Gram matrix Gj (268 MMAC via PE transposes + 32 accumulation matmuls)
and a handful of 256x256 matmuls. 1/N is split as (1/64)*(1/64) across
the two host factors to stay inside fp16 normal range.

Device pipeline (one batch element per core, 8 cores):
  phase G  transpose xj chunks with the PE (fp16 transpose = 53ns/tile),
           accumulate Gj in PSUM; sxj rowsum on DVE (fp16 2x mode).
  phase 2  t1 = Gj AG64; V = Pt64 t1 (+ two rank-1 corrections);
           E'^T = T^T V + I;  d = V^T t_b + W_b.
  phase 3  z0 tiles [128, 512] = E_aug^T.T @ xi (d folded into the BN
           affine, NOT added here); DVE bn_stats per tile.
  BN       per-channel (mean, mean-of-squares)/8 for both chunks in ONE
           AllGather (out [8, 128, 4]) + local 3-add reduction.
  stores   normalize (a*z0 - b_eff) in 512-col pieces, DVE/ACT split,
           each piece DMA'd as soon as it is ready.

Compute dtype fp16 (PE streams fp16 at 1 cycle/row; 10 mantissa bits keep
the end-to-end rms relative error ~1e-3 vs the fp32 reference).
"""

import numpy as np

import concourse.bass as bass
import concourse.bacc as bacc
import concourse.tile as tile
from concourse import mybir
from concourse import bass_utils

B = 8
C = 256
N = 4096          # 64 * 64
NCORES = 8
NCH = 2           # channel chunks of 128
NT = 32           # n chunks of 128 (phase G)
NZ = 8            # n tiles of 512 (phase 3)
NQJ = 8           # xj DMA pieces (512 cols each)
NQI = 4           # xi DMA pieces (1024 cols each)
F32 = mybir.dt.float32
FP16 = mybir.dt.float16
BN_EPS = 1e-5

MM_DT = FP16


def build_kernel(nc, skip_cc: bool = False) -> None:
    f32 = F32
    xi_d = nc.dram_tensor("xi", [C, N], MM_DT, kind="ExternalInput").ap()
    xj_d = nc.dram_tensor("xj", [C, N], MM_DT, kind="ExternalInput").ap()
    # [128, 128]: identity (transpose permutation operand)
    idt_d = nc.dram_tensor("idt", [128, 128], MM_DT, kind="ExternalInput").ap()
    # [128, 2, 256]: (G^T W^T)/64, chunked on the contraction index
    ag64_d = nc.dram_tensor("ag64", [128, NCH, C], MM_DT, kind="ExternalInput").ap()
    # [128, 2, 256]: P^T/64, chunked on the contraction index
    pt64_d = nc.dram_tensor("pt64", [128, NCH, C], MM_DT, kind="ExternalInput").ap()
    # [128, 2, 256]: (P^T T)/64, chunked on the contraction index
    tp64_d = nc.dram_tensor("tp64", [128, NCH, C], MM_DT, kind="ExternalInput").ap()
    # [128, 2, 256]: theta_w rows, chunked (for tvrow)
    wtw_d = nc.dram_tensor("wtw", [128, NCH, C], MM_DT, kind="ExternalInput").ap()
    # [128, 2]: theta_b column, chunked
    wtb_d = nc.dram_tensor("wtb", [128, NCH], MM_DT, kind="ExternalInput").ap()
    # [128, 2]: (P t_b)/64 column, chunked
    qtb_d = nc.dram_tensor("qtb", [128, NCH], MM_DT, kind="ExternalInput").ap()
    # [1, 3*256+64]: rows [W g_b | (W g_b)/64 | T^T p_b | consts]
    aux_d = nc.dram_tensor("aux", [1, 3 * C + 64], MM_DT, kind="ExternalInput").ap()
    # [128, 2, 2]: (gamma, beta) per channel, chunked
    gbe_d = nc.dram_tensor("gbe", [128, NCH, 2], f32, kind="ExternalInput").ap()
    # [128, 2]: W_b column, chunked
    wbc_d = nc.dram_tensor("wbc", [128, NCH], f32, kind="ExternalInput").ap()
    # [128, 2, 256]: identity matrix chunks (for E'^T + I)
    idn_d = nc.dram_tensor("idn", [128, NCH, C], MM_DT, kind="ExternalInput").ap()
    out_d = nc.dram_tensor("out", [C, N], f32, kind="ExternalOutput").ap()

    with tile.TileContext(nc) as tc:
        _body(tc, xi_d, xj_d, idt_d, ag64_d, pt64_d, tp64_d, wtw_d, wtb_d,
              qtb_d, aux_d, gbe_d, idn_d, wbc_d, out_d, skip_cc=skip_cc)


def _body(tc, xi_d, xj_d, idt_d, ag64_d, pt64_d, tp64_d, wtw_d, wtb_d,
          qtb_d, aux_d, gbe_d, idn_d, wbc_d, out_d, skip_cc: bool = False):
    nc = tc.nc
    f32 = F32
    import contextlib

    with contextlib.ExitStack() as ctx:
        constp = ctx.enter_context(tc.tile_pool(name="const", bufs=1))
        datap = ctx.enter_context(tc.tile_pool(name="data", bufs=1))
        workp = ctx.enter_context(tc.tile_pool(name="work", bufs=4))
        rowsp = ctx.enter_context(tc.tile_pool(name="rows", bufs=2))
        psbig = ctx.enter_context(tc.tile_pool(name="ps_big", bufs=4, space="PSUM"))
        psacc = ctx.enter_context(tc.tile_pool(name="ps_acc", bufs=2, space="PSUM"))
        pssml = ctx.enter_context(tc.tile_pool(name="ps_sml", bufs=2, space="PSUM"))
        dramp = ctx.enter_context(tc.tile_pool(name="dram", bufs=2, space="DRAM"))

        mdt = MM_DT
        # ---- loads: transpose identity first, then the data streams.
        # Sync queue (HWDGE): no Pool desc-gen serialization.
        idt = constp.tile([128, 128], mdt, tag="idt")
        nc.sync.dma_start(out=idt, in_=idt_d)
        JW = N // NQJ
        xj_h = []
        for h in range(NQJ):
            t = datap.tile([128, NCH, JW], mdt, tag=f"xjh{h}", name=f"xj_h{h}")
            nc.sync.dma_start(
                out=t,
                in_=xj_d.rearrange("(k p) n -> p k n", p=128)[:, :, h * JW:(h + 1) * JW],
            )
            xj_h.append(t)
        IW = N // NQI
        xi_h = []
        for h in range(NQI):
            t = datap.tile([128, NCH, IW], mdt, tag=f"xih{h}", name=f"xi_h{h}")
            nc.sync.dma_start(
                out=t,
                in_=xi_d.rearrange("(k p) n -> p k n", p=128)[:, :, h * IW:(h + 1) * IW],
            )
            xi_h.append(t)
        # ---- phase-2 weights and small constants: also on the sync queue
        # (after the big loads) so the Pool engine has NO desc-gen work and
        # is free to run the sxj reduction
        ag64 = constp.tile([128, NCH, C], mdt, tag="ag64")
        nc.sync.dma_start(out=ag64, in_=ag64_d)
        pt64 = constp.tile([128, NCH, C], mdt, tag="pt64")
        nc.sync.dma_start(out=pt64, in_=pt64_d)
        tp64 = constp.tile([128, NCH, C], mdt, tag="tp64")
        nc.sync.dma_start(out=tp64, in_=tp64_d)
        w_tw = constp.tile([128, NCH, C], mdt, tag="w_tw")
        nc.sync.dma_start(out=w_tw, in_=wtw_d)
        w_tb = constp.tile([128, NCH], mdt, tag="w_tb")
        nc.sync.dma_start(out=w_tb, in_=wtb_d)
        qtb = constp.tile([128, NCH], mdt, tag="qtb")
        nc.sync.dma_start(out=qtb, in_=qtb_d)
        aux = constp.tile([1, 3 * C + 64], mdt, tag="aux")
        nc.sync.dma_start(out=aux, in_=aux_d)
        idn = constp.tile([128, NCH, C], mdt, tag="idn")
        nc.sync.dma_start(out=idn, in_=idn_d)
        gbe = constp.tile([128, NCH, 2], f32, tag="gbe")
        nc.sync.dma_start(out=gbe, in_=gbe_d)
        wbc = constp.tile([128, NCH], f32, tag="wbc")
        nc.sync.dma_start(out=wbc, in_=wbc_d)
        wgbrow = aux[:, 0:C]
        wgb64row = aux[:, C:2 * C]
        tpbrow = aux[:, 2 * C:3 * C]    # (T^T p_b)^T
        c64 = aux[:, 3 * C:3 * C + 1]   # 1/64
        cpt = aux[:, 3 * C + 1:3 * C + 2]   # p_b . t_b
        eps = constp.tile([128, 1], f32, tag="eps")
        nc.vector.memset(eps, BN_EPS)
        # preload both activation-function tables (Identity for the copies,
        # Sqrt for the BN tail) while the ACT engine is still idle
        warm = rowsp.tile([128, 1], f32, tag="warm")
        nc.scalar.activation(
            out=warm, in_=eps, func=mybir.ActivationFunctionType.Identity,
            bias=eps, scale=1.0,
        )
        nc.scalar.activation(
            out=warm, in_=eps, func=mybir.ActivationFunctionType.Sqrt,
            bias=eps, scale=1.0,
        )
        # warm-up matmuls: keep the PE busy from t~0.5us so the p-state ramp
        # (full clock only after 3us of continuous execution) completes
        # before the real work arrives
        warm64 = constp.tile([128, 64], f32, tag="warm64")
        nc.vector.memset(warm64, 0.0)
        warm_ps = pssml.tile([1, 64], f32, tag="sml", name="warm_ps")
        for _ in range(14):
            nc.tensor.matmul(warm_ps, eps, warm64, start=True, stop=True)

        def xi_sl(k, tix):
            # phase-3 tile tix of 512 columns, channel-chunk k
            h, off = divmod(tix * 512, IW)
            return xi_h[h][:, k, off:off + 512]

        def xj_sl(k, i):
            # chunk i of 128 columns, channel-chunk k
            h, off = divmod(i * 128, JW)
            return xj_h[h][:, k, off:off + 128]

        # ---- phase G: Gj = Xj Xj^T via PE transposes -----------------
        # software-pipelined two chunks deep: transposes for chunk i+2 are
        # emitted BEFORE the Gram accumulation of chunk i so the in-order PE
        # queue never stalls on the PSUM->SBUF copy of the transposed tile.
        # Each transposed tile carries an extra ones column, so the Gram's
        # moving dim is 257 and its last column delivers sxj = Xj @ 1 for
        # free -- no separate rowsum machinery at all.
        CA = C + 1
        Gj_ps = [psacc.tile([128, CA], f32, tag="acc", name=f"Gj_ps{m}")
                 for m in range(NCH)]
        tis = []

        def trans(i):
            # transpose PSUM staging reuses the [128,512]f32 "big" bank pool
            # via a bitcast view, so one pool serves both phase G and phase 3
            t_raw = psbig.tile([128, 512], f32, tag="big", name=f"tps{i}")
            t_ps = t_raw.bitcast(MM_DT)[:, 0:C]
            for k in range(NCH):
                nc.tensor.transpose(
                    t_ps[:, k * 128:(k + 1) * 128], xj_sl(k, i), idt)
            ti = workp.tile([128, CA], mdt, tag="ti", name=f"ti{i}")
            nc.vector.tensor_copy(ti[:, 0:C], t_ps)
            nc.vector.memset(ti[:, C:CA], 1.0)
            tis.append(ti)

        r1_ps = pssml.tile([1, C], f32, tag="sml", name="r1_ps")
        uprow = rowsp.tile([1, C], mdt, tag="uprow")

        def sxj_col(k):
            return Gj_sb[k][:, C:CA]

        def emit_rank1_rows():
            # r1 = ((W G)/64 sxj)^T;  u'row = r1/64 + (W g_b)^T
            for k in range(NCH):
                nc.tensor.matmul(
                    r1_ps, sxj_col(k), ag64[:, k, :],
                    start=(k == 0), stop=(k == NCH - 1),
                )
            nc.vector.scalar_tensor_tensor(
                out=uprow, in0=r1_ps, scalar=c64, in1=wgbrow,
                op0=mybir.AluOpType.mult, op1=mybir.AluOpType.add,
            )

        trans(0)
        trans(1)
        for i in range(NT):
            if i + 2 < NT:
                trans(i + 2)
            ti = tis[i]
            for m in range(NCH):
                nc.tensor.matmul(
                    Gj_ps[m],
                    ti[:, m * 128:(m + 1) * 128],
                    ti,
                    start=(i == 0), stop=(i == NT - 1),
                )
        Gj_sb = []
        for m in range(NCH):
            t = workp.tile([128, CA], mdt, tag=f"Gj{m}")
            if m == 0:
                nc.vector.tensor_copy(t, Gj_ps[m])
            else:
                nc.scalar.copy(t, Gj_ps[m])
            Gj_sb.append(t)

        # ---- phase 2: t1 = Gj AG64; E'^T = TP64 t1 + rank-1s --------
        # V never materializes on the critical chain: E'^T = T^T V is
        # computed directly via the host-folded TP64 = (P^T T)/64, and the
        # d column is rebuilt from t1 during the phase-3 j=0 window.
        # small rank-1 ingredients first (need only the sxj columns of Gja)
        t1_ps = []
        for m in range(NCH):
            t_ps = psacc.tile([128, C], f32, tag="acc", name=f"t1_ps{m}")
            msl = slice(m * 128, (m + 1) * 128)
            for k in range(NCH):
                nc.tensor.matmul(
                    t_ps, Gj_sb[k][:, msl], ag64[:, k, :],
                    start=(k == 0), stop=(k == NCH - 1),
                )
            t1_ps.append(t_ps)
        # rank-1 row matmuls overlap the t1 PSUM->SBUF copies below
        emit_rank1_rows()
        v64c_ps = pssml.tile([128, NCH], f32, tag="sml", name="v64c_ps")
        for c2 in range(NCH):
            for k in range(NCH):
                nc.tensor.matmul(
                    v64c_ps[:, c2:c2 + 1],
                    pt64[:, k, c2 * 128:(c2 + 1) * 128],
                    sxj_col(k),
                    start=(k == 0), stop=(k == NCH - 1),
                )
        v64col = rowsp.tile([128, NCH], mdt, tag="v64col")
        nc.vector.tensor_copy(v64col, v64c_ps)
        # tvrow = (T^T v64)^T ; svt = v64 . t_b  (for the d column later)
        tv_ps = pssml.tile([1, C + 1], f32, tag="sml", name="tv_ps")
        for c2 in range(NCH):
            nc.tensor.matmul(
                tv_ps[:, 0:C], v64col[:, c2:c2 + 1], w_tw[:, c2, :],
                start=(c2 == 0), stop=(c2 == NCH - 1),
            )
        for c2 in range(NCH):
            nc.tensor.matmul(
                tv_ps[:, C:C + 1], v64col[:, c2:c2 + 1], w_tb[:, c2:c2 + 1],
                start=(c2 == 0), stop=(c2 == NCH - 1),
            )
        tvrow = rowsp.tile([1, C + 1], mdt, tag="tvrow")
        nc.vector.tensor_copy(tvrow, tv_ps)
        t1_sb = []
        for m in range(NCH):
            t = workp.tile([128, C], mdt, tag=f"t1{m}")
            if m == 0:
                nc.vector.tensor_copy(t, t1_ps[m])
            else:
                nc.scalar.copy(t, t1_ps[m])
            t1_sb.append(t)
        ET_sb = []
        for m in range(NCH):
            e_ps = psacc.tile([128, C], f32, tag="acc")
            msl = slice(m * 128, (m + 1) * 128)
            for k in range(NCH):
                nc.tensor.matmul(
                    e_ps, tp64[:, k, msl], t1_sb[k],
                    start=(k == 0), stop=False,
                )
            # E'^T += (T^T p_b) (u')^T + (T^T v64) (W g_b / 64)^T
            nc.tensor.matmul(
                e_ps, tpbrow[:, msl], uprow, start=False, stop=False)
            nc.tensor.matmul(
                e_ps, tvrow[:, msl], wgb64row, start=False, stop=True)
            t = workp.tile([128, C], mdt, tag=f"ET{m}")
            nc.vector.tensor_add(t, e_ps, idn[:, m, :])
            ET_sb.append(t)

        # ---- phase 3: z0 = (E'+I)^T.T @ xi; BN stats fused ----------
        z_t = datap.tile([128, NCH, N], f32, tag="z")
        spack = rowsp.tile([128, 4], f32, tag="spack")
        cc_in = dramp.tile([128, 4], f32, tag="cc_in", name="cc_in")
        cc_out = dramp.tile([NCORES, 128, 4], f32, tag="cc_out", name="cc_out")
        dcol_ps = pssml.tile([128, NCH], f32, tag="sml")
        dcol = rowsp.tile([128, NCH], f32, tag="dcol")
        for j in range(NCH):
            jsl = slice(j * 128, (j + 1) * 128)
            stats = workp.tile([128, NZ, 6], f32, tag="bnst", name=f"stats{j}")
            for tix in range(NZ):
                tsl = slice(tix * 512, (tix + 1) * 512)
                z_ps = psbig.tile([128, 512], f32, tag="big")
                for k in range(NCH):
                    nc.tensor.matmul(
                        z_ps, ET_sb[k][:, jsl], xi_sl(k, tix),
                        start=(k == 0), stop=(k == NCH - 1),
                    )
                nc.scalar.copy(z_t[:, j, tsl], z_ps)
                nc.vector.bn_stats(out=stats[:, tix, :], in_=z_t[:, j, tsl])
            if j == 0:
                # d column = V^T t_b + W_b rebuilt from t1 (V is never
                # materialized), off the phase-3 critical entry:
                # d = t1^T (P^T t_b)/64 + uprow^T (p_b.t_b)
                #     + wgb64^T (v64.t_b) + W_b
                for jj in range(NCH):
                    jjsl = slice(jj * 128, (jj + 1) * 128)
                    for k in range(NCH):
                        nc.tensor.matmul(
                            dcol_ps[:, jj:jj + 1],
                            t1_sb[k][:, jjsl],
                            qtb[:, k:k + 1],
                            start=(k == 0), stop=False,
                        )
                    nc.tensor.matmul(
                        dcol_ps[:, jj:jj + 1], uprow[:, jjsl], cpt,
                        start=False, stop=False,
                    )
                    nc.tensor.matmul(
                        dcol_ps[:, jj:jj + 1], wgb64row[:, jjsl],
                        tvrow[:, C:C + 1],
                        start=False, stop=True,
                    )
                nc.vector.tensor_add(dcol, dcol_ps, wbc)
            mv = rowsp.tile([128, 2], f32, tag="mv")
            nc.vector.bn_aggr(out=mv, in_=stats)
            # true mean = mean(z0) + d;  spack = (mean/8, (mean^2+var)/8)
            mt = rowsp.tile([128, 1], f32, tag="mt")
            nc.vector.tensor_add(mt, mv[:, 0:1], dcol[:, j:j + 1])
            nc.vector.tensor_scalar_mul(
                spack[:, 2 * j:2 * j + 1], mt, 1.0 / NCORES)
            nc.vector.scalar_tensor_tensor(
                out=spack[:, 2 * j + 1:2 * j + 2], in0=mt,
                scalar=mt, in1=mv[:, 1:2],
                op0=mybir.AluOpType.mult, op1=mybir.AluOpType.add,
            )
            nc.vector.tensor_scalar_mul(
                spack[:, 2 * j + 1:2 * j + 2],
                spack[:, 2 * j + 1:2 * j + 2], 1.0 / NCORES)
            # stage this chunk's stats to DRAM immediately: chunk 0's DMA
            # overlaps chunk 1's compute, only chunk 1's is exposed
            nc.sync.dma_start(
                out=cc_in[:, 2 * j:2 * j + 2], in_=spack[:, 2 * j:2 * j + 2])

        # ---- ONE AllGather for both chunks' stats; local 8-way sum --
        if skip_cc:
            nc.sync.dma_start(out=cc_out[0, :, :], in_=cc_in)
        else:
            nc.gpsimd.collective_compute(
                "AllGather",
                mybir.AluOpType.bypass,
                replica_groups=[list(range(NCORES))],
                ins=[cc_in.opt()],
                outs=[cc_out.opt()],
            )
        sall = rowsp.tile([128, NCORES, 4], f32, tag="sall")
        nc.sync.dma_start(
            out=sall, in_=cc_out.rearrange("r p s -> p r s"))
        s4 = rowsp.tile([128, 4, 4], f32, tag="s4")
        nc.vector.tensor_add(s4, sall[:, 0:4, :], sall[:, 4:8, :])
        s2 = rowsp.tile([128, 2, 4], f32, tag="s2")
        nc.vector.tensor_add(s2, s4[:, 0:2, :], s4[:, 2:4, :])
        ssum = rowsp.tile([128, 4], f32, tag="ssum")
        nc.vector.tensor_add(ssum, s2[:, 0, :], s2[:, 1, :])

        # ---- normalize + affine + store -----------------------------
        # affine vectors for BOTH chunks at once on [128, 2] strided views
        mcols = ssum[:, 0:4:2]
        qcols = ssum[:, 1:4:2]
        # negvar = m^2 - q  (sqrt uses scale=-1 to flip the sign)
        nv2 = rowsp.tile([128, 2], f32, tag="nv2")
        nc.vector.tensor_mul(nv2, mcols, mcols)
        nc.vector.tensor_sub(nv2, nv2, qcols)
        # nm0 = d - m = -(global mean of z0); independent of the sqrt chain
        nm02 = rowsp.tile([128, 2], f32, tag="nm02")
        nc.vector.tensor_sub(nm02, dcol, mcols)
        # sc = sqrt(var + eps);  a = gamma / sc
        sc2 = rowsp.tile([128, 2], f32, tag="sc2")
        nc.scalar.activation(
            out=sc2, in_=nv2, func=mybir.ActivationFunctionType.Sqrt,
            bias=eps, scale=-1.0,
        )
        nc.vector.reciprocal(out=sc2, in_=sc2)
        ac2 = rowsp.tile([128, 2], f32, tag="ac2")
        nc.vector.tensor_mul(ac2, sc2, gbe[:, :, 0])
        # nb = beta - a*(m - d):  out = a*z0 + nb
        nb2 = rowsp.tile([128, 2], f32, tag="nb2")
        nc.vector.tensor_mul(nb2, nm02, ac2)
        nc.vector.tensor_add(nb2, nb2, gbe[:, :, 1])
        # normalize in 512-col pieces, DVE and ACT alternating, store each
        # piece as soon as it is ready so the output DMA pipeline starts
        # right after the collective
        PW = N // 8
        for j in range(NCH):
            acol = ac2[:, j:j + 1]
            nbcol = nb2[:, j:j + 1]
            for p in range(8):
                psl = slice(p * PW, (p + 1) * PW)
                if p % 2 == 0:
                    nc.vector.tensor_scalar(
                        out=z_t[:, j, psl], in0=z_t[:, j, psl],
                        scalar1=acol, scalar2=nbcol,
                        op0=mybir.AluOpType.mult, op1=mybir.AluOpType.add,
                    )
                else:
                    nc.scalar.activation(
                        out=z_t[:, j, psl], in_=z_t[:, j, psl],
                        func=mybir.ActivationFunctionType.Identity,
                        bias=nbcol, scale=acol,
                    )
                nc.sync.dma_start(
                    out=out_d[j * 128:(j + 1) * 128, psl], in_=z_t[:, j, psl])


_NC_CACHE: dict = {}


def _get_nc():
    if "nc" not in _NC_CACHE:
        nc = bacc.Bacc(
            "TRN2",
            target_bir_lowering=False,
            debug=False,
            enable_asserts=True,
            num_devices=NCORES,
        )
        build_kernel(nc)
        nc.compile()
        _NC_CACHE["nc"] = nc
    return _NC_CACHE["nc"]


def _make_in_maps(inputs: dict) -> list[dict]:
    f16 = np.float16
    xi = np.ascontiguousarray(
        np.asarray(inputs["xi"], np.float32).reshape(B, C, N).astype(f16))
    xj = np.ascontiguousarray(
        np.asarray(inputs["xj"], np.float32).reshape(B, C, N).astype(f16))
    g_w = np.asarray(inputs["g_w"], np.float32)
    g_b = np.asarray(inputs["g_b"], np.float32)
    t_w = np.asarray(inputs["theta_w"], np.float32)
    t_b = np.asarray(inputs["theta_b"], np.float32)
    p_w = np.asarray(inputs["phi_w"], np.float32)
    p_b = np.asarray(inputs["phi_b"], np.float32)
    W_w = np.asarray(inputs["W_w"], np.float32)
    W_b = np.asarray(inputs["W_b"], np.float32)
    gam = np.asarray(inputs["bn_gamma"], np.float32)
    bet = np.asarray(inputs["bn_beta"], np.float32)

    def chunked(a):  # [256, F] -> [128, 2, F]
        return np.ascontiguousarray(a.reshape(2, 128, -1).transpose(1, 0, 2))

    idt = np.eye(128, dtype=np.float32).astype(f16)                # [128,128]
    ag64 = chunked(g_w.T @ W_w.T / 64.0).astype(f16)               # [128,2,256]
    pt64 = chunked(p_w.T / 64.0).astype(f16)                       # [128,2,256]
    tp64 = chunked(p_w.T @ t_w / 64.0).astype(f16)                 # [128,2,256]
    wtw = chunked(t_w).astype(f16)                                 # [128,2,256]
    wtb = np.ascontiguousarray(t_b.reshape(2, 128).T).astype(f16)  # [128,2]
    qtb = np.ascontiguousarray(
        (p_w.T @ t_b / 64.0).reshape(2, 128).T).astype(f16)        # [128,2]
    wgb = W_w @ g_b                                                # [256]
    consts = np.zeros(64, np.float32)
    consts[0] = 1.0 / 64.0
    consts[1] = float(p_b @ t_b)
    aux = np.concatenate([wgb, wgb / 64.0, t_w.T @ p_b, consts])[None, :]
    aux = np.ascontiguousarray(aux.astype(f16))                    # [1,832]
    gbe = chunked(np.stack([gam, bet], axis=1))                    # [128,2,2]
    idn = chunked(np.eye(C, dtype=np.float32)).astype(f16)         # [128,2,256]
    wbc = np.ascontiguousarray(W_b.reshape(2, 128).T)              # [128,2]

    in_maps = []
    for b in range(B):
        in_maps.append({
            "xi": xi[b], "xj": xj[b],
            "idt": idt, "ag64": ag64, "pt64": pt64, "tp64": tp64,
            "wtw": wtw, "wtb": wtb, "qtb": qtb,
            "aux": aux, "gbe": gbe, "idn": idn, "wbc": wbc,
        })
    return in_maps


def kernel(**inputs) -> np.ndarray:
    nc = _get_nc()
    in_maps = _make_in_maps(inputs)
    last_err = None
    for attempt in range(3):
        try:
            res = bass_utils.run_bass_kernel_spmd(
                nc, in_maps, core_ids=list(range(NCORES)),
            )
            break
        except Exception as e:  # transient device wedge: back off and retry
            last_err = e
            import time as _time
            _time.sleep(4.0 * (attempt + 1))
            try:
                import jax
                import jax.extend.backend as _jeb
                jax.clear_caches()
                # tear down the PJRT client: a fresh axon connection lets the
                # terminal reset a wedged exec unit
                _jeb.clear_backends()
            except Exception:
                pass
    else:
        raise last_err
    out = np.stack([res.results[c]["out"] for c in range(NCORES)])
    return np.ascontiguousarray(out.reshape(B, C, 64, 64).astype(np.float32))


if __name__ == "__main__":
    rng = np.random.default_rng(0)
    fake = {
        "xi": rng.standard_normal((B, C, 64, 64), np.float32),
        "xj": rng.standard_normal((B, C, 64, 64), np.float32),
        "g_w": rng.standard_normal((C, C), np.float32) / 16,
        "g_b": rng.standard_normal((C,), np.float32) / 16,
        "theta_w": rng.standard_normal((C, C), np.float32) / 16,
        "theta_b": rng.standard_normal((C,), np.float32) / 16,
        "phi_w": rng.standard_normal((C, C), np.float32) / 16,
        "phi_b": rng.standard_normal((C,), np.float32) / 16,
        "W_w": rng.standard_normal((C, C), np.float32) / 16,
        "W_b": rng.standard_normal((C,), np.float32) / 16,
        "bn_gamma": np.ones((C,), np.float32),
        "bn_beta": np.zeros((C,), np.float32),
    }
    out = kernel(**fake)
    print("out", out.shape, out.dtype, float(np.abs(out).mean()))
